# revision 1
# baseline (speedup 1.0000x reference)
"""Trainium2 Bass kernel for nn_EntropyLoss (retrieval_knn).

Math: per (l,b) sample x = feats[l,b].reshape(C, H*W), the heavy part is the
C x C gram matrix over D = H*W = 65536.  Everything after the gram (pairwise
distances, 7th-smallest selection, per-layer sums, log, variance) touches only
C*C = 4096 values per sample and runs on host, replicating the fp32 reference
arithmetic.

Active design (v3: `build_kernel_bf` + `pack_xt_bf`, data-parallel, 3
samples/core):
  - The PE contracts along partitions, so matmul operands need d on
    partitions.  The host pre-tiles each sample into its transposed SBUF
    image (no on-device transposes), split as x = hi + lo (bf16 pair, exact
    to ~2^-17): slabs [128, 4096] bf16 where chunk w occupies columns
    [w*128, w*128+128) = [hi(64 c) | lo(64 c)] of d = g*4096 + w*128 + p.
    Each slab is one fully-contiguous 1 MiB DMA.
  - Per 128-chunk ONE bf16 matmul [K=128, M=64, N=128]: stationary hi,
    moving [hi|lo] -> accumulates A += hi^T hi and B += hi^T lo side by side
    at 1 cycle/row (4x cheaper than fp32).  Chunks alternate (w % 2) between
    partition halves of a [128, 128] gram PSUM tile (PE column groups 0/64
    run concurrently).
  - Host reconstructs G = A + B + B^T (lo^T hi = B^T) and adds the exact
    diag(lo^T lo) computed on host; the omitted off-diag lo^T lo is O(0.02),
    ~100x inside the fp32 result's ULP-robustness envelope (verified
    bit-exact vs the reference on multiple inputs).

`build_kernel_mm` (v2, fp32 pure-matmul, ~1.8x slower) and `build_kernel`
(v1, all-on-device with PE transposes, ~3.5x slower) kept for fallback.
"""

from collections import deque

import numpy as np

C = 64            # channels (gram is C x C)
PAIR = 128        # contraction chunk per matmul (PE partition limit)
GROUP_PAIRS = 4   # chunk-pairs per PSUM staging bank -> [128, 512] fp32
SLAB_F = 4096     # free columns per DMA slab ([128, 4096] tile, 2 MiB)
PIPE_DELAY = 2    # bank-groups between transpose emission and MM consumption

N_CORES = 8
L, B, HW = 3, 8, 65536
SAMPLES = L * B
S_PER_CORE = SAMPLES // N_CORES


def build_kernel(n_samples: int, D: int, repeat: int = 1, tpsum_bufs: int = 5,
                 stage_bufs: int = 3, xt_bufs: int = 5, pipe_delay: int = PIPE_DELAY):
    from concourse import bacc
    import concourse.mybir as mybir
    import concourse.tile as tile

    assert n_samples in (2, 3)
    fp32 = mybir.dt.float32
    nc = bacc.Bacc("TRN2", target_bir_lowering=False, debug=False)

    n_out = 1 if n_samples == 2 else 2
    xs = nc.dram_tensor("xs", [2, C, D], fp32, kind="ExternalInput")
    if n_samples == 3:
        xsolo = nc.dram_tensor("xsolo", [2 * C, D // 2], fp32, kind="ExternalInput")
    g2 = nc.dram_tensor("g2", [n_out, 2 * C, C], fp32, kind="ExternalOutput")

    ident_np = np.eye(2 * C, dtype=np.float32)
    ident = nc.inline_tensor(ident_np, name="ident128")

    bank_cols = 2 * GROUP_PAIRS * C  # 512
    groups_per_slab = SLAB_F // (PAIR * GROUP_PAIRS)  # 8

    with tile.TileContext(nc) as tc:
        with (
            tc.tile_pool(name="consts", bufs=1) as const_pool,
            tc.tile_pool(name="stage", bufs=stage_bufs) as stage_pool,
            tc.tile_pool(name="tpsum", bufs=tpsum_bufs, space="PSUM") as tpsum_pool,
            tc.tile_pool(name="xT", bufs=xt_bufs) as xT_pool,
            tc.tile_pool(name="gpsum", bufs=2, space="PSUM") as gpsum_pool,
            tc.tile_pool(name="outs", bufs=2) as out_pool,
        ):
            id_sb = const_pool.tile_from(ident[:])  # [128, 128]

            def run_phase(out_slot, slab_srcs):
                """slab_srcs: list of [128, SLAB_F] APs; low/high partition
                halves accumulate into partitions 0:64 / 64:128 of g_ps."""
                n_slabs = len(slab_srcs)
                mm_per_half = n_slabs * SLAB_F // PAIR
                g_ps = gpsum_pool.tile([2 * C, C], fp32)
                mm_count = [0, 0]
                pending = deque()

                def emit_mms(xT):
                    for k in range(2 * GROUP_PAIRS):
                        h = k % 2
                        cnt = mm_count[h]
                        tile_ap = xT[:, k * C:(k + 1) * C]
                        nc.tensor.matmul(
                            out=g_ps[h * C:(h + 1) * C, :],
                            lhsT=tile_ap,
                            rhs=tile_ap,
                            start=(cnt == 0),
                            stop=(cnt == mm_per_half - 1),
                            skip_group_check=True,
                        )
                        mm_count[h] += 1

                for src in slab_srcs:
                    stage = stage_pool.tile([2 * C, SLAB_F], fp32)
                    nc.sync.dma_start(stage[:], src)
                    for b in range(groups_per_slab):
                        ps = tpsum_pool.tile([PAIR, bank_cols], fp32)
                        for j in range(GROUP_PAIRS):
                            jj = b * GROUP_PAIRS + j
                            # [128,128] full-partition transpose: output cols
                            # 0:64 = low half's chunk, 64:128 = high half's.
                            nc.tensor.transpose(
                                ps[:, j * PAIR:(j + 1) * PAIR],
                                stage[:, jj * PAIR:(jj + 1) * PAIR],
                                id_sb[:],
                            )
                        xT = xT_pool.tile([PAIR, bank_cols], fp32)
                        if b % 2 == 0:
                            nc.vector.tensor_copy(xT, ps)
                        else:
                            nc.scalar.copy(xT, ps)
                        pending.append(xT)
                        if len(pending) > pipe_delay:
                            emit_mms(pending.popleft())
                while pending:
                    emit_mms(pending.popleft())

                g2_sb = out_pool.tile([2 * C, C], fp32)
                nc.vector.tensor_copy(g2_sb, g_ps)
                nc.sync.dma_start(g2[out_slot], g2_sb)

            # Phase 1: samples 0 and 1 stacked on partitions, one slab per
            # SLAB_F columns.  Output slot 0 = [gram(s0); gram(s1)].
            pair_srcs = [
                xs[0:2, :, w * SLAB_F:(w + 1) * SLAB_F].rearrange(
                    "s c f -> (s c) f"
                )
                for w in range(D // SLAB_F)
            ]
            solo_srcs = [
                xsolo[:, u * SLAB_F:(u + 1) * SLAB_F]
                for u in range(D // 2 // SLAB_F)
            ] if n_samples == 3 else None

            # repeat > 1 re-runs the whole computation (benchmarking only;
            # outputs are simply rewritten).
            for _ in range(repeat):
                run_phase(0, pair_srcs)
                # Phase 2: sample 2 (host-restacked to [128, D/2]).  Output
                # slot 1 = [half_gram_A; half_gram_B], summed on host.
                if n_samples == 3:
                    run_phase(1, solo_srcs)

    nc.compile()
    return nc


def build_kernel_mm(n_samples: int, D: int, repeat: int = 1,
                    slab_bufs: int = 4, slab_f: int = SLAB_F):
    """Pure-matmul kernel: host supplies pre-tiled transposed data.

    Input xt: [n_samples, n_slabs, 128, slab_f], packed by `pack_xt` so that
    xt[s, g, p, w*C + c] = x[s, c, d] with d = g*(slab_f//C*128) + w*128 + p.
    Each [128, slab_f] slab is one full-bandwidth contiguous DMA; every
    C-column slice is a ready matmul operand [K=128, 64].  Chunks alternate
    (w % 2) between partition halves 0:64 / 64:128 of the gram PSUM tile
    (PE column groups 0/64 run concurrently); host folds the two half-grams.
    """
    from concourse import bacc
    import concourse.mybir as mybir
    import concourse.tile as tile

    fp32 = mybir.dt.float32
    nc = bacc.Bacc("TRN2", target_bir_lowering=False, debug=False)

    chunks_per_slab = slab_f // C
    n_slabs = D // (chunks_per_slab * PAIR)
    xt = nc.dram_tensor(
        "xt", [n_samples, n_slabs, PAIR, slab_f], fp32, kind="ExternalInput"
    )
    g2 = nc.dram_tensor("g2", [n_samples, 2 * C, C], fp32, kind="ExternalOutput")

    with tile.TileContext(nc) as tc:
        with (
            tc.tile_pool(name="slab", bufs=slab_bufs) as slab_pool,
            tc.tile_pool(name="gpsum", bufs=2, space="PSUM") as gpsum_pool,
            tc.tile_pool(name="outs", bufs=2) as out_pool,
        ):
            for _ in range(repeat):
                for s in range(n_samples):
                    g_ps = gpsum_pool.tile([2 * C, C], fp32)
                    mm_count = [0, 0]
                    mm_per_half = n_slabs * chunks_per_slab // 2
                    for g in range(n_slabs):
                        slab = slab_pool.tile([PAIR, slab_f], fp32)
                        nc.sync.dma_start(slab[:], xt[s, g])
                        for w in range(chunks_per_slab):
                            h = w % 2
                            cnt = mm_count[h]
                            tap = slab[:, w * C:(w + 1) * C]
                            nc.tensor.matmul(
                                out=g_ps[h * C:(h + 1) * C, :],
                                lhsT=tap,
                                rhs=tap,
                                start=(cnt == 0),
                                stop=(cnt == mm_per_half - 1),
                                skip_group_check=True,
                            )
                            mm_count[h] += 1
                    g2_sb = out_pool.tile([2 * C, C], fp32)
                    nc.vector.tensor_copy(g2_sb, g_ps)
                    nc.sync.dma_start(g2[s], g2_sb)

    nc.compile()
    return nc


def pack_xt(x: np.ndarray, slab_f: int = SLAB_F) -> np.ndarray:
    """x: [n_samples, C, D] -> [n_samples, n_slabs, 128, slab_f] pre-tiled
    transposed layout: xt[s, g, p, w*C + c] = x[s, c, g*(2*slab_f) + w*128 + p]."""
    ns, c, d = x.shape
    per_slab_d = slab_f // C * PAIR
    n_slabs = d // per_slab_d
    v = x.reshape(ns, c, n_slabs, slab_f // C, PAIR)  # (s, c, g, w, p)
    return np.ascontiguousarray(v.transpose(0, 2, 4, 3, 1)).reshape(
        ns, n_slabs, PAIR, slab_f
    )


_KERNEL_CACHE = {}


def _get_kernel(n_samples: int, D: int):
    key = ("mm", n_samples, D)
    if key not in _KERNEL_CACHE:
        _KERNEL_CACHE[key] = build_kernel_mm(n_samples, D)
    return _KERNEL_CACHE[key]


def _get_kernel_bf(n_samples: int, D: int):
    key = ("bf", n_samples, D)
    if key not in _KERNEL_CACHE:
        # Interleaving the 3 samples' slab streams removes the per-sample
        # accumulation-drain boundaries: 53.4 vs 71.8 us/iter same-session.
        _KERNEL_CACHE[key] = build_kernel_bf(
            n_samples, D, interleave=True, slab_bufs=5
        )
    return _KERNEL_CACHE[key]


def grams_from_g2(g2: np.ndarray, n_cores: int = N_CORES) -> np.ndarray:
    """g2 (v1 layout): [n_cores, 2, 128, 64] -> grams [3*n_cores, 64, 64]."""
    grams = np.zeros((3 * n_cores, C, C), dtype=np.float32)
    for i in range(n_cores):
        grams[3 * i + 0] = g2[i, 0, :C, :]
        grams[3 * i + 1] = g2[i, 0, C:, :]
        grams[3 * i + 2] = g2[i, 1, :C, :] + g2[i, 1, C:, :]
    return grams


def grams_from_g2_mm(g2: np.ndarray) -> np.ndarray:
    """g2 (v2 layout): [n_total_samples, 128, 64] half-gram pairs."""
    return (g2[:, :C, :] + g2[:, C:, :]).astype(np.float32)


def _postprocess(grams: np.ndarray):
    """grams: [SAMPLES, C, C] fp32 -> scalar, replicating reference fp32 math."""
    K = C // 10
    rballs = np.zeros((SAMPLES, C), dtype=np.float32)
    for i in range(SAMPLES):
        g = grams[i]
        sq = np.diagonal(g).copy()
        d2 = (sq[:, None] + sq[None, :]) - np.float32(2.0) * g
        d2 = np.clip(d2, np.float32(1e-8), None)
        dist = np.sqrt(d2, dtype=np.float32)
        rballs[i] = np.sort(dist, axis=-1)[:, K]

    rb = rballs.reshape(L, B * C)
    try:
        import jax

        cpu = jax.devices("cpu")[0]
        with jax.default_device(cpu):
            import jax.numpy as jnp

            H = jnp.sum(jnp.asarray(rb), axis=-1)
            ent = jnp.log(H + 1.0)
            delta = ent[1:] - ent[:-1]
            var = jnp.var(delta, ddof=1)
            return np.asarray(var, dtype=np.float32)
    except Exception:
        H = rb.astype(np.float32).sum(axis=-1)
        ent = np.log(H + np.float32(1.0)).astype(np.float32)
        delta = ent[1:] - ent[:-1]
        n = delta.shape[0]
        mean = np.float32(delta.mean())
        var = np.float32(((delta - mean) ** 2).sum() / np.float32(n - 1))
        return np.asarray(var, dtype=np.float32)


def kernel(feats: np.ndarray) -> np.ndarray:
    from concourse.bass_utils import run_bass_kernel_spmd

    feats = np.ascontiguousarray(feats, dtype=np.float32)
    x = feats.reshape(SAMPLES, C, HW)

    nc = _get_kernel_bf(S_PER_CORE, HW)
    packs = [
        pack_xt_bf(x[i * S_PER_CORE:(i + 1) * S_PER_CORE])
        for i in range(N_CORES)
    ]
    in_maps = [{"xt": p[0]} for p in packs]
    sqlo = np.concatenate([p[1] for p in packs], axis=0)  # [24, 64]
    res = run_bass_kernel_spmd(nc, in_maps, core_ids=list(range(N_CORES)))
    g2 = np.concatenate([r["g2"] for r in res.results], axis=0)  # [24,128,128]
    grams = grams_from_g2_bf(g2, sqlo)
    return _postprocess(grams)


if __name__ == "__main__":
    feats = np.random.default_rng(0).standard_normal(
        (L, B, C, 256, 256)
    ).astype(np.float32)
    print(kernel(feats))


# ---------------------------------------------------------------------------
# v3: split-precision bf16 kernel.  x = hi + lo (both bf16).  One bf16 matmul
# per 128-chunk with stationary hi and moving [hi | lo] (n=128, 1 cyc/row on
# the PE vs fp32's 4) accumulates A = hi^T hi and B = hi^T lo side by side;
# host reconstructs G = A + B + B^T (lo^T hi = B^T) and adds the exact
# diagonal of lo^T lo (computed on host) — the only omitted term's off-diag
# is O(0.02), far inside the fp32-rounding robustness envelope.
# ---------------------------------------------------------------------------

BF_SLAB_F = 4096  # bf16 columns per slab: 32 chunks x [hi(64) | lo(64)]


def build_kernel_bf(n_samples: int, D: int, repeat: int = 1,
                    slab_bufs: int = 4, slab_f: int = BF_SLAB_F,
                    dma_alt: bool = False, interleave: bool = False):
    from concourse import bacc
    import concourse.mybir as mybir
    import concourse.tile as tile

    fp32 = mybir.dt.float32
    bf16 = mybir.dt.bfloat16
    nc = bacc.Bacc("TRN2", target_bir_lowering=False, debug=False)

    chunks_per_slab = slab_f // PAIR
    n_slabs = D // (chunks_per_slab * PAIR)
    xt = nc.dram_tensor(
        "xt", [n_samples, n_slabs, PAIR, slab_f], bf16, kind="ExternalInput"
    )
    g2 = nc.dram_tensor(
        "g2", [n_samples, PAIR, PAIR], fp32, kind="ExternalOutput"
    )

    with tile.TileContext(nc) as tc:
        with (
            tc.tile_pool(name="slab", bufs=slab_bufs) as slab_pool,
            tc.tile_pool(
                name="gpsum", bufs=(n_samples + 1 if interleave else 2),
                space="PSUM",
            ) as gpsum_pool,
            tc.tile_pool(name="outs", bufs=2) as out_pool,
        ):
            mm_per_half = n_slabs * chunks_per_slab // 2

            def do_slab(slab_src, g_ps, mm_count):
                slab = slab_pool.tile([PAIR, slab_f], bf16)
                nc.sync.dma_start(slab[:], slab_src)
                for w in range(chunks_per_slab):
                    h = w % 2
                    cnt = mm_count[h]
                    hi = slab[:, w * PAIR:w * PAIR + C]
                    hilo = slab[:, w * PAIR:(w + 1) * PAIR]
                    nc.tensor.matmul(
                        out=g_ps[h * C:(h + 1) * C, :],
                        lhsT=hi,
                        rhs=hilo,
                        start=(cnt == 0),
                        stop=(cnt == mm_per_half - 1),
                        skip_group_check=True,
                    )
                    mm_count[h] += 1

            def finish(s, g_ps):
                g2_sb = out_pool.tile([PAIR, PAIR], fp32)
                nc.vector.tensor_copy(g2_sb, g_ps)
                nc.sync.dma_start(g2[s], g2_sb)

            for _ in range(repeat):
                if interleave:
                    g_tiles = [
                        gpsum_pool.tile([PAIR, PAIR], fp32, name=f"gps{si}",
                                        tag="g")
                        for si in range(n_samples)
                    ]
                    counts = [[0, 0] for _ in range(n_samples)]
                    for g in range(n_slabs):
                        for s in range(n_samples):
                            do_slab(xt[s, g], g_tiles[s], counts[s])
                    for s in range(n_samples):
                        finish(s, g_tiles[s])
                else:
                    for s in range(n_samples):
                        g_ps = gpsum_pool.tile([PAIR, PAIR], fp32)
                        mm_count = [0, 0]
                        for g in range(n_slabs):
                            do_slab(xt[s, g], g_ps, mm_count)
                        finish(s, g_ps)

    nc.compile()
    return nc


def pack_xt_bf(x: np.ndarray, slab_f: int = BF_SLAB_F):
    """x: [ns, C, D] fp32 -> (xtb [ns, n_slabs, 128, BF_SLAB_F] bf16,
    sqlo [ns, C] fp32: exact sum of lo^2 per channel).

    Layout: xtb[s, g, p, w*128 + c]        = bf16 hi of x[s, c, d]
            xtb[s, g, p, w*128 + 64 + c]   = bf16 lo of x[s, c, d]
    with d = g*4096 + w*128 + p.
    """
    import ml_dtypes

    bf = ml_dtypes.bfloat16
    ns, c, d = x.shape
    hi = x.astype(bf)
    lo32 = x - hi.astype(np.float32)
    lo = lo32.astype(bf)
    sqlo = (lo.astype(np.float64) ** 2).sum(axis=-1).astype(np.float32)

    n_slabs = d // slab_f
    out = np.empty((ns, n_slabs, PAIR, slab_f // PAIR, 2, C), dtype=bf)
    vh = hi.reshape(ns, c, n_slabs, slab_f // PAIR, PAIR)
    vl = lo.reshape(ns, c, n_slabs, slab_f // PAIR, PAIR)
    out[:, :, :, :, 0, :] = vh.transpose(0, 2, 4, 3, 1)
    out[:, :, :, :, 1, :] = vl.transpose(0, 2, 4, 3, 1)
    return out.reshape(ns, n_slabs, PAIR, slab_f), sqlo


def grams_from_g2_bf(g2: np.ndarray, sqlo: np.ndarray) -> np.ndarray:
    """g2: [n, 128, 128] (A|B on each partition half) -> grams [n, 64, 64]."""
    n = g2.shape[0]
    grams = np.zeros((n, C, C), dtype=np.float32)
    for i in range(n):
        A = g2[i, :C, :C] + g2[i, C:, :C]
        Bm = g2[i, :C, C:] + g2[i, C:, C:]
        G = A + Bm + Bm.T
        G[np.arange(C), np.arange(C)] += sqlo[i]
        grams[i] = G
    return grams



# revision 3
# speedup vs baseline: 2.8571x; 2.8571x over previous
"""Trainium2 Bass kernel for nn_EntropyLoss (retrieval_knn).

Math: per (l,b) sample x = feats[l,b].reshape(C, H*W), the heavy part is the
C x C gram matrix over D = H*W = 65536.  Everything after the gram (pairwise
distances, 7th-smallest selection, per-layer sums, log, variance) touches only
C*C = 4096 values per sample and runs on host, replicating the fp32 reference
arithmetic.

Active design (v3: `build_kernel_bf` + `pack_xt_bf`, data-parallel, 3
samples/core):
  - The PE contracts along partitions, so matmul operands need d on
    partitions.  The host pre-tiles each sample into its transposed SBUF
    image (no on-device transposes), split as x = hi + lo (bf16 pair, exact
    to ~2^-17): slabs [128, 4096] bf16 where chunk w occupies columns
    [w*128, w*128+128) = [hi(64 c) | lo(64 c)] of d = g*4096 + w*128 + p.
    Each slab is one fully-contiguous 1 MiB DMA.
  - Per 128-chunk ONE bf16 matmul [K=128, M=64, N=128]: stationary hi,
    moving [hi|lo] -> accumulates A += hi^T hi and B += hi^T lo side by side
    at 1 cycle/row (4x cheaper than fp32).  Chunks alternate (w % 2) between
    partition halves of a [128, 128] gram PSUM tile (PE column groups 0/64
    run concurrently).
  - Host reconstructs G = A + B + B^T (lo^T hi = B^T) and adds the exact
    diag(lo^T lo) computed on host; the omitted off-diag lo^T lo is O(0.02),
    ~100x inside the fp32 result's ULP-robustness envelope (verified
    bit-exact vs the reference on multiple inputs).

`build_kernel_mm` (v2, fp32 pure-matmul, ~1.8x slower) and `build_kernel`
(v1, all-on-device with PE transposes, ~3.5x slower) kept for fallback.
"""

from collections import deque

import numpy as np

C = 64            # channels (gram is C x C)
PAIR = 128        # contraction chunk per matmul (PE partition limit)
GROUP_PAIRS = 4   # chunk-pairs per PSUM staging bank -> [128, 512] fp32
SLAB_F = 4096     # free columns per DMA slab ([128, 4096] tile, 2 MiB)
PIPE_DELAY = 2    # bank-groups between transpose emission and MM consumption

N_CORES = 8
L, B, HW = 3, 8, 65536
SAMPLES = L * B
S_PER_CORE = SAMPLES // N_CORES


def build_kernel(n_samples: int, D: int, repeat: int = 1, tpsum_bufs: int = 5,
                 stage_bufs: int = 3, xt_bufs: int = 5, pipe_delay: int = PIPE_DELAY):
    from concourse import bacc
    import concourse.mybir as mybir
    import concourse.tile as tile

    assert n_samples in (2, 3)
    fp32 = mybir.dt.float32
    nc = bacc.Bacc("TRN2", target_bir_lowering=False, debug=False)

    n_out = 1 if n_samples == 2 else 2
    xs = nc.dram_tensor("xs", [2, C, D], fp32, kind="ExternalInput")
    if n_samples == 3:
        xsolo = nc.dram_tensor("xsolo", [2 * C, D // 2], fp32, kind="ExternalInput")
    g2 = nc.dram_tensor("g2", [n_out, 2 * C, C], fp32, kind="ExternalOutput")

    ident_np = np.eye(2 * C, dtype=np.float32)
    ident = nc.inline_tensor(ident_np, name="ident128")

    bank_cols = 2 * GROUP_PAIRS * C  # 512
    groups_per_slab = SLAB_F // (PAIR * GROUP_PAIRS)  # 8

    with tile.TileContext(nc) as tc:
        with (
            tc.tile_pool(name="consts", bufs=1) as const_pool,
            tc.tile_pool(name="stage", bufs=stage_bufs) as stage_pool,
            tc.tile_pool(name="tpsum", bufs=tpsum_bufs, space="PSUM") as tpsum_pool,
            tc.tile_pool(name="xT", bufs=xt_bufs) as xT_pool,
            tc.tile_pool(name="gpsum", bufs=2, space="PSUM") as gpsum_pool,
            tc.tile_pool(name="outs", bufs=2) as out_pool,
        ):
            id_sb = const_pool.tile_from(ident[:])  # [128, 128]

            def run_phase(out_slot, slab_srcs):
                """slab_srcs: list of [128, SLAB_F] APs; low/high partition
                halves accumulate into partitions 0:64 / 64:128 of g_ps."""
                n_slabs = len(slab_srcs)
                mm_per_half = n_slabs * SLAB_F // PAIR
                g_ps = gpsum_pool.tile([2 * C, C], fp32)
                mm_count = [0, 0]
                pending = deque()

                def emit_mms(xT):
                    for k in range(2 * GROUP_PAIRS):
                        h = k % 2
                        cnt = mm_count[h]
                        tile_ap = xT[:, k * C:(k + 1) * C]
                        nc.tensor.matmul(
                            out=g_ps[h * C:(h + 1) * C, :],
                            lhsT=tile_ap,
                            rhs=tile_ap,
                            start=(cnt == 0),
                            stop=(cnt == mm_per_half - 1),
                            skip_group_check=True,
                        )
                        mm_count[h] += 1

                for src in slab_srcs:
                    stage = stage_pool.tile([2 * C, SLAB_F], fp32)
                    nc.sync.dma_start(stage[:], src)
                    for b in range(groups_per_slab):
                        ps = tpsum_pool.tile([PAIR, bank_cols], fp32)
                        for j in range(GROUP_PAIRS):
                            jj = b * GROUP_PAIRS + j
                            # [128,128] full-partition transpose: output cols
                            # 0:64 = low half's chunk, 64:128 = high half's.
                            nc.tensor.transpose(
                                ps[:, j * PAIR:(j + 1) * PAIR],
                                stage[:, jj * PAIR:(jj + 1) * PAIR],
                                id_sb[:],
                            )
                        xT = xT_pool.tile([PAIR, bank_cols], fp32)
                        if b % 2 == 0:
                            nc.vector.tensor_copy(xT, ps)
                        else:
                            nc.scalar.copy(xT, ps)
                        pending.append(xT)
                        if len(pending) > pipe_delay:
                            emit_mms(pending.popleft())
                while pending:
                    emit_mms(pending.popleft())

                g2_sb = out_pool.tile([2 * C, C], fp32)
                nc.vector.tensor_copy(g2_sb, g_ps)
                nc.sync.dma_start(g2[out_slot], g2_sb)

            # Phase 1: samples 0 and 1 stacked on partitions, one slab per
            # SLAB_F columns.  Output slot 0 = [gram(s0); gram(s1)].
            pair_srcs = [
                xs[0:2, :, w * SLAB_F:(w + 1) * SLAB_F].rearrange(
                    "s c f -> (s c) f"
                )
                for w in range(D // SLAB_F)
            ]
            solo_srcs = [
                xsolo[:, u * SLAB_F:(u + 1) * SLAB_F]
                for u in range(D // 2 // SLAB_F)
            ] if n_samples == 3 else None

            # repeat > 1 re-runs the whole computation (benchmarking only;
            # outputs are simply rewritten).
            for _ in range(repeat):
                run_phase(0, pair_srcs)
                # Phase 2: sample 2 (host-restacked to [128, D/2]).  Output
                # slot 1 = [half_gram_A; half_gram_B], summed on host.
                if n_samples == 3:
                    run_phase(1, solo_srcs)

    nc.compile()
    return nc


def build_kernel_mm(n_samples: int, D: int, repeat: int = 1,
                    slab_bufs: int = 4, slab_f: int = SLAB_F):
    """Pure-matmul kernel: host supplies pre-tiled transposed data.

    Input xt: [n_samples, n_slabs, 128, slab_f], packed by `pack_xt` so that
    xt[s, g, p, w*C + c] = x[s, c, d] with d = g*(slab_f//C*128) + w*128 + p.
    Each [128, slab_f] slab is one full-bandwidth contiguous DMA; every
    C-column slice is a ready matmul operand [K=128, 64].  Chunks alternate
    (w % 2) between partition halves 0:64 / 64:128 of the gram PSUM tile
    (PE column groups 0/64 run concurrently); host folds the two half-grams.
    """
    from concourse import bacc
    import concourse.mybir as mybir
    import concourse.tile as tile

    fp32 = mybir.dt.float32
    nc = bacc.Bacc("TRN2", target_bir_lowering=False, debug=False)

    chunks_per_slab = slab_f // C
    n_slabs = D // (chunks_per_slab * PAIR)
    xt = nc.dram_tensor(
        "xt", [n_samples, n_slabs, PAIR, slab_f], fp32, kind="ExternalInput"
    )
    g2 = nc.dram_tensor("g2", [n_samples, 2 * C, C], fp32, kind="ExternalOutput")

    with tile.TileContext(nc) as tc:
        with (
            tc.tile_pool(name="slab", bufs=slab_bufs) as slab_pool,
            tc.tile_pool(name="gpsum", bufs=2, space="PSUM") as gpsum_pool,
            tc.tile_pool(name="outs", bufs=2) as out_pool,
        ):
            for _ in range(repeat):
                for s in range(n_samples):
                    g_ps = gpsum_pool.tile([2 * C, C], fp32)
                    mm_count = [0, 0]
                    mm_per_half = n_slabs * chunks_per_slab // 2
                    for g in range(n_slabs):
                        slab = slab_pool.tile([PAIR, slab_f], fp32)
                        nc.sync.dma_start(slab[:], xt[s, g])
                        for w in range(chunks_per_slab):
                            h = w % 2
                            cnt = mm_count[h]
                            tap = slab[:, w * C:(w + 1) * C]
                            nc.tensor.matmul(
                                out=g_ps[h * C:(h + 1) * C, :],
                                lhsT=tap,
                                rhs=tap,
                                start=(cnt == 0),
                                stop=(cnt == mm_per_half - 1),
                                skip_group_check=True,
                            )
                            mm_count[h] += 1
                    g2_sb = out_pool.tile([2 * C, C], fp32)
                    nc.vector.tensor_copy(g2_sb, g_ps)
                    nc.sync.dma_start(g2[s], g2_sb)

    nc.compile()
    return nc


def pack_xt(x: np.ndarray, slab_f: int = SLAB_F) -> np.ndarray:
    """x: [n_samples, C, D] -> [n_samples, n_slabs, 128, slab_f] pre-tiled
    transposed layout: xt[s, g, p, w*C + c] = x[s, c, g*(2*slab_f) + w*128 + p]."""
    ns, c, d = x.shape
    per_slab_d = slab_f // C * PAIR
    n_slabs = d // per_slab_d
    v = x.reshape(ns, c, n_slabs, slab_f // C, PAIR)  # (s, c, g, w, p)
    return np.ascontiguousarray(v.transpose(0, 2, 4, 3, 1)).reshape(
        ns, n_slabs, PAIR, slab_f
    )


_KERNEL_CACHE = {}


def _get_kernel(n_samples: int, D: int):
    key = ("mm", n_samples, D)
    if key not in _KERNEL_CACHE:
        _KERNEL_CACHE[key] = build_kernel_mm(n_samples, D)
    return _KERNEL_CACHE[key]


def _get_kernel_bf(n_samples: int, D: int):
    key = ("bf", n_samples, D)
    if key not in _KERNEL_CACHE:
        # Interleaving the 3 samples' slab streams removes the per-sample
        # accumulation-drain boundaries: 53.4 vs 71.8 us/iter same-session.
        _KERNEL_CACHE[key] = build_kernel_bf(
            n_samples, D, interleave=True, slab_bufs=8
        )
    return _KERNEL_CACHE[key]


def grams_from_g2(g2: np.ndarray, n_cores: int = N_CORES) -> np.ndarray:
    """g2 (v1 layout): [n_cores, 2, 128, 64] -> grams [3*n_cores, 64, 64]."""
    grams = np.zeros((3 * n_cores, C, C), dtype=np.float32)
    for i in range(n_cores):
        grams[3 * i + 0] = g2[i, 0, :C, :]
        grams[3 * i + 1] = g2[i, 0, C:, :]
        grams[3 * i + 2] = g2[i, 1, :C, :] + g2[i, 1, C:, :]
    return grams


def grams_from_g2_mm(g2: np.ndarray) -> np.ndarray:
    """g2 (v2 layout): [n_total_samples, 128, 64] half-gram pairs."""
    return (g2[:, :C, :] + g2[:, C:, :]).astype(np.float32)


def _postprocess(grams: np.ndarray):
    """grams: [SAMPLES, C, C] fp32 -> scalar, replicating reference fp32 math."""
    K = C // 10
    rballs = np.zeros((SAMPLES, C), dtype=np.float32)
    for i in range(SAMPLES):
        g = grams[i]
        sq = np.diagonal(g).copy()
        d2 = (sq[:, None] + sq[None, :]) - np.float32(2.0) * g
        d2 = np.clip(d2, np.float32(1e-8), None)
        dist = np.sqrt(d2, dtype=np.float32)
        rballs[i] = np.sort(dist, axis=-1)[:, K]

    rb = rballs.reshape(L, B * C)
    try:
        import jax

        cpu = jax.devices("cpu")[0]
        with jax.default_device(cpu):
            import jax.numpy as jnp

            H = jnp.sum(jnp.asarray(rb), axis=-1)
            ent = jnp.log(H + 1.0)
            delta = ent[1:] - ent[:-1]
            var = jnp.var(delta, ddof=1)
            return np.asarray(var, dtype=np.float32)
    except Exception:
        H = rb.astype(np.float32).sum(axis=-1)
        ent = np.log(H + np.float32(1.0)).astype(np.float32)
        delta = ent[1:] - ent[:-1]
        n = delta.shape[0]
        mean = np.float32(delta.mean())
        var = np.float32(((delta - mean) ** 2).sum() / np.float32(n - 1))
        return np.asarray(var, dtype=np.float32)


def kernel(feats: np.ndarray) -> np.ndarray:
    from concourse.bass_utils import run_bass_kernel_spmd

    feats = np.ascontiguousarray(feats, dtype=np.float32)
    x = feats.reshape(SAMPLES, C, HW)

    nc = _get_kernel_bf(S_PER_CORE, HW)
    packs = [
        pack_xt_bf(x[i * S_PER_CORE:(i + 1) * S_PER_CORE])
        for i in range(N_CORES)
    ]
    in_maps = [{"xt": p[0]} for p in packs]
    sqlo = np.concatenate([p[1] for p in packs], axis=0)  # [24, 64]
    res = run_bass_kernel_spmd(nc, in_maps, core_ids=list(range(N_CORES)))
    g2 = np.concatenate([r["g2"] for r in res.results], axis=0)  # [24,128,128]
    grams = grams_from_g2_bf(g2, sqlo)
    return _postprocess(grams)


if __name__ == "__main__":
    feats = np.random.default_rng(0).standard_normal(
        (L, B, C, 256, 256)
    ).astype(np.float32)
    print(kernel(feats))


# ---------------------------------------------------------------------------
# v3: split-precision bf16 kernel.  x = hi + lo (both bf16).  One bf16 matmul
# per 128-chunk with stationary hi and moving [hi | lo] (n=128, 1 cyc/row on
# the PE vs fp32's 4) accumulates A = hi^T hi and B = hi^T lo side by side;
# host reconstructs G = A + B + B^T (lo^T hi = B^T) and adds the exact
# diagonal of lo^T lo (computed on host) — the only omitted term's off-diag
# is O(0.02), far inside the fp32-rounding robustness envelope.
# ---------------------------------------------------------------------------

BF_SLAB_F = 8192  # bf16 columns per slab: 64 chunks x [hi(64) | lo(64)]
# 2 MiB per slab DMA (vs 1 MiB at 4096): halves DMA-instruction/semaphore
# count; with slab_bufs=8 the prefetch window is 16 MiB, decoupling PE from
# DMA-completion latency jitter.  The packed byte stream and the chunk ->
# PSUM-half assignment/order are unchanged (chunks_per_slab stays even), so
# the fp32 accumulation sequence -- and hence the result -- is bit-identical
# to the slab_f=4096 layout.


def build_kernel_bf(n_samples: int, D: int, repeat: int = 1,
                    slab_bufs: int = 4, slab_f: int = BF_SLAB_F,
                    dma_alt: bool = False, interleave: bool = False):
    from concourse import bacc
    import concourse.mybir as mybir
    import concourse.tile as tile

    fp32 = mybir.dt.float32
    bf16 = mybir.dt.bfloat16
    nc = bacc.Bacc("TRN2", target_bir_lowering=False, debug=False)

    chunks_per_slab = slab_f // PAIR
    n_slabs = D // (chunks_per_slab * PAIR)
    xt = nc.dram_tensor(
        "xt", [n_samples, n_slabs, PAIR, slab_f], bf16, kind="ExternalInput"
    )
    g2 = nc.dram_tensor(
        "g2", [n_samples, PAIR, PAIR], fp32, kind="ExternalOutput"
    )

    with tile.TileContext(nc) as tc:
        with (
            tc.tile_pool(name="slab", bufs=slab_bufs) as slab_pool,
            tc.tile_pool(
                name="gpsum", bufs=(n_samples + 1 if interleave else 2),
                space="PSUM",
            ) as gpsum_pool,
            tc.tile_pool(name="outs", bufs=2) as out_pool,
        ):
            mm_per_half = n_slabs * chunks_per_slab // 2

            def do_slab(slab_src, g_ps, mm_count):
                slab = slab_pool.tile([PAIR, slab_f], bf16)
                nc.sync.dma_start(slab[:], slab_src)
                for w in range(chunks_per_slab):
                    h = w % 2
                    cnt = mm_count[h]
                    hi = slab[:, w * PAIR:w * PAIR + C]
                    hilo = slab[:, w * PAIR:(w + 1) * PAIR]
                    nc.tensor.matmul(
                        out=g_ps[h * C:(h + 1) * C, :],
                        lhsT=hi,
                        rhs=hilo,
                        start=(cnt == 0),
                        stop=(cnt == mm_per_half - 1),
                        skip_group_check=True,
                    )
                    mm_count[h] += 1

            def finish(s, g_ps):
                g2_sb = out_pool.tile([PAIR, PAIR], fp32)
                nc.vector.tensor_copy(g2_sb, g_ps)
                nc.sync.dma_start(g2[s], g2_sb)

            for _ in range(repeat):
                if interleave:
                    g_tiles = [
                        gpsum_pool.tile([PAIR, PAIR], fp32, name=f"gps{si}",
                                        tag="g")
                        for si in range(n_samples)
                    ]
                    counts = [[0, 0] for _ in range(n_samples)]
                    for g in range(n_slabs):
                        for s in range(n_samples):
                            do_slab(xt[s, g], g_tiles[s], counts[s])
                    for s in range(n_samples):
                        finish(s, g_tiles[s])
                else:
                    for s in range(n_samples):
                        g_ps = gpsum_pool.tile([PAIR, PAIR], fp32)
                        mm_count = [0, 0]
                        for g in range(n_slabs):
                            do_slab(xt[s, g], g_ps, mm_count)
                        finish(s, g_ps)

    nc.compile()
    return nc


def pack_xt_bf(x: np.ndarray, slab_f: int = BF_SLAB_F):
    """x: [ns, C, D] fp32 -> (xtb [ns, n_slabs, 128, BF_SLAB_F] bf16,
    sqlo [ns, C] fp32: exact sum of lo^2 per channel).

    Layout: xtb[s, g, p, w*128 + c]        = bf16 hi of x[s, c, d]
            xtb[s, g, p, w*128 + 64 + c]   = bf16 lo of x[s, c, d]
    with d = g*4096 + w*128 + p.
    """
    import ml_dtypes

    bf = ml_dtypes.bfloat16
    ns, c, d = x.shape
    hi = x.astype(bf)
    lo32 = x - hi.astype(np.float32)
    lo = lo32.astype(bf)
    sqlo = (lo.astype(np.float64) ** 2).sum(axis=-1).astype(np.float32)

    n_slabs = d // slab_f
    out = np.empty((ns, n_slabs, PAIR, slab_f // PAIR, 2, C), dtype=bf)
    vh = hi.reshape(ns, c, n_slabs, slab_f // PAIR, PAIR)
    vl = lo.reshape(ns, c, n_slabs, slab_f // PAIR, PAIR)
    out[:, :, :, :, 0, :] = vh.transpose(0, 2, 4, 3, 1)
    out[:, :, :, :, 1, :] = vl.transpose(0, 2, 4, 3, 1)
    return out.reshape(ns, n_slabs, PAIR, slab_f), sqlo


def grams_from_g2_bf(g2: np.ndarray, sqlo: np.ndarray) -> np.ndarray:
    """g2: [n, 128, 128] (A|B on each partition half) -> grams [n, 64, 64]."""
    n = g2.shape[0]
    grams = np.zeros((n, C, C), dtype=np.float32)
    for i in range(n):
        A = g2[i, :C, :C] + g2[i, C:, :C]
        Bm = g2[i, :C, C:] + g2[i, C:, C:]
        G = A + Bm + Bm.T
        G[np.arange(C), np.arange(C)] += sqlo[i]
        grams[i] = G
    return grams

